# revision 1
# baseline (speedup 1.0000x reference)
"""Trainium2 Bass kernel for nn_DMPNet_76012331205204.

The reference runs a 500-step DMP (dynamic movement primitive) scan after a
2-layer MLP. The scan is linear in its per-element state (y, z), so the whole
rollout collapses exactly into

    y[i, t, d] = A[t]*y0[i,d] + Cst[t] + gy0[i,d] * Z2[i, (t,d)]
    Z2 = feat[i] @ WG[:, (t,d)] + bias(t,d)   (WG = W_last cols folded with G)
    gy0 = goal - y0,  goal = feat @ W_last[:, :7] + b_last[:7]

with G computed on the host in float64 from c, h. t=0 is y0 exactly and is
filled host-side; the device computes t=1..50 (NQ=350 output cols per row).

Device dataflow per core (batch 512 = 4 tiles of 128):
  - all tensors fp16 (PE full rate at any moving size; half the HBM bytes;
    fp16's 11-bit mantissa matches the f32r rounding the PE applies anyway)
  - b_pt folds into the MLP matmul as an extra ones-row of xT (contraction 66)
  - A[t]*y0+Cst ("pa") is precomputed on the HOST and DMA'd, which removes
    the s2 matmul and makes the final DVE add all-fp16/SBUF (2x_1p rate)
  - 3 packed input DMAs (wx | pa | wc+sy+ly), 2 packed output stores, all on
    HWDGE with fresh queues so every instruction keeps a single sync-wait
  - per tile: 3 PE matmuls -> PSUM; DVE gy copy (walrus allows one PSUM
    operand per DVE op), DVE prod (1x), DVE add (2x) -> fp16 SBUF -> store

Batch 4096 sharded 512/core across 8 cores, no cross-core communication.
"""

import numpy as np

import bass_rust as _bass_rust

import concourse.bass as bass
import concourse.tile as tile
from concourse import mybir
from concourse.bass_utils import run_bass_kernel_spmd
from concourse.vector_clock import ScopedClock


class _SplitDrainTileContext(tile.TileContext):
    """TileContext whose kernel-tail drain carries at most one sync-wait.

    The walrus build in this container rejects instructions with more than
    one sync-wait command ("Too many sync wait commands"). Tile's exit-time
    drain waits on every outstanding semaphore at once; spread those waits
    over a chain of single-wait SP nops instead (SP executes in order, so
    the drain still happens after everything it must wait for).
    """

    def _drain_and_barrier(self, tick_clock, wait_clock):
        probe = self.nc.sync.nop(hint="tail_wait", nofuse=True)
        wait_clock.add_sem_waits(
            probe.ins, ScopedClock({None: tick_clock.global_clock}))
        waits = list(probe.ins.sync_info.on_wait or []) if probe.ins.sync_info else []
        if len(waits) > 1:
            probe.ins.sync_info.on_wait = waits[:1]
            for w in waits[1:]:
                n = self.nc.sync.nop(hint="tail_wait", nofuse=True)
                n.ins.sync_info = _bass_rust.SyncInfo(on_wait=[w], on_update=[])
        self.nc.sync.drain()
        self.nc.all_engine_barrier()
        assert self.sems is not None
        popped = self.nc._tile_sem_poison_stack.pop()
        assert popped is self._sem_poison
        self.nc.clear_and_free_semaphores(list(self.sems.allocated().values()))
        self.nc.gpsimd.drain()


# Problem constants (hardcoded per contract; kernel.py must be self-contained)
N = 30
T = 50
L = 10
TAU = 1.0
A_Z = 15.0
A_X = 1.0
DOF = 7
SCALE = 1.0
DT = TAU / (T * L)
STEPS = T * L                # 500
B = 4096
D_IN = 64
HID = 256
NCORES = 8
BS = B // NCORES             # 512 batch rows per core
NT = STEPS // L + 1          # 51 output time points
NQ = (NT - 1) * DOF          # 350 device-computed cols (t=1..50)
NC_MAIN = DOF + NQ + 1       # 358 cols of the fused output matmul (even)
KA = 66                      # contraction: 64 + b_pt ones-row + pad

_F32 = mybir.dt.float32
_F16 = mybir.dt.float16

# wcl pack column offsets: [wc0 | wc1 | sy | lyT]
_WC0 = 0
_WC1 = NC_MAIN               # 358
_SY = 2 * NC_MAIN            # 716
_LY = 3 * NC_MAIN            # 1074
_WCL_W = _LY + BS            # 1586


def _precompute_coeffs(c, h):
    """Collapse the linear scan: returns (G [NT,N], coef_goal, A, Cst) f64."""
    c = np.asarray(c, np.float64)
    h = np.asarray(h, np.float64)
    b_z = A_Z / 4.0
    xs = np.empty(STEPS)
    xv = 1.0
    for t in range(STEPS):
        xv = xv + (-A_X * xv / TAU) * DT
        xs[t] = xv
    psi = np.exp(-h[None, :] * (xs[:, None] - c[None, :]) ** 2)     # [STEPS, N]
    phi = psi * (xs / psi.sum(1))[:, None]                          # [STEPS, N]

    M = np.array([[1.0, DT / TAU], [-DT * A_Z * b_z / TAU, 1.0 - DT * A_Z / TAU]])
    Mp = np.empty((STEPS + 1, 2, 2))
    Mp[0] = np.eye(2)
    for i in range(1, STEPS + 1):
        Mp[i] = M @ Mp[i - 1]

    out_ts = range(0, STEPS + 1, L)
    coef_y0 = np.array([Mp[t][0, 0] for t in out_ts])
    coef_z0 = np.array([Mp[t][0, 1] for t in out_ts])
    coef_goal = np.empty(NT)
    G = np.zeros((NT, N))
    for j, Tt in enumerate(out_ts):
        ks = Mp[Tt - 1 :: -1, 0, 1][:Tt] if Tt > 0 else np.zeros(0)
        coef_goal[j] = (DT * A_Z * b_z / TAU) * ks.sum()
        if Tt > 0:
            G[j] = (DT / TAU) * (ks[:, None] * phi[:Tt]).sum(0)
    A = coef_y0 + coef_goal          # multiplies y0
    Cst = coef_z0 * 0.05 * TAU       # constant (z0 = 0.05*TAU)
    return G, coef_goal, A, Cst


def _build_nc():
    """One-core SPMD program; all 8 cores run it on their batch shard."""
    nc = bass.Bass("TRN2", target_bir_lowering=False, debug=False,
                   num_devices=NCORES)
    wxa_d = nc.dram_tensor("wxa_s", [KA, HID + 256], _F16,
                           kind="ExternalInput")
    wxb_d = nc.dram_tensor("wxb_s", [KA, 256], _F16, kind="ExternalInput")
    wcl_d = nc.dram_tensor("wcl_s", [128, _WCL_W], _F16,
                           kind="ExternalInput")
    pa_d = nc.dram_tensor("pa_s", [128, 4 * NQ], _F16, kind="ExternalInput")
    y_d = nc.dram_tensor("y", [BS, NQ], _F16, kind="ExternalOutput")

    with _SplitDrainTileContext(nc) as tc:
        with (
            tc.tile_pool(name="const", bufs=1) as cpool,
            tc.tile_pool(name="work", bufs=2) as wpool,
            tc.tile_pool(name="psf", bufs=4, space="PSUM") as psf,
            tc.tile_pool(name="psm", bufs=1, space="PSUM") as psm,
        ):
            one_sb = wpool.tile([1, 1], _F32, tag="one_sb")
            nc.vector.memset(one_sb[:], 1.0)
            # pm23's tail cols are never read: park the PE p-state warmup
            # transpose there (saves a PSUM bank).
            pm23 = psm.tile([128, 1024], _F32, tag="pm23")
            nc.tensor.transpose(pm23[0:1, 1008:1024][:, 0:1], one_sb[:],
                                one_sb[:])
            # ACT function-table prefetch (~1.3us) during the DMA-wait head.
            aabs = wpool.tile([1, 1], _F32, tag="aabs")
            nc.scalar.activation(aabs[:], one_sb[:],
                                 mybir.ActivationFunctionType.Tanh)

            # 4 input DMAs on fresh HWDGE queues, critical-path first:
            # wxa (wpt + first xT half) unblocks the MLP; wcl unblocks the
            # fused matmuls; pa is only needed by the final adds.
            wxa = cpool.tile([KA, HID + 256], _F16)
            nc.sync.dma_start(wxa[:], wxa_d[:])
            wxb = cpool.tile([KA, 256], _F16)
            nc.sync.dma_start(wxb[:], wxb_d[:])
            wcl = cpool.tile([128, _WCL_W], _F16)
            nc.scalar.dma_start(wcl[:], wcl_d[:])
            pa_sb = cpool.tile([128, 4 * NQ], _F16)
            nc.sync.dma_start(pa_sb[:], pa_d[:])

            # featT [256, BS] = tanh(W_pt_aug.T @ xT_aug), fp16. Batch
            # chunks of 128/128/256 cols with a rotating PSUM tile per
            # (m, chunk) so each tanh waits only on its own matmul (Tile
            # deps are tile-granular) and tile b0's combine starts early.
            ft0 = cpool.tile([128, BS], _F16, tag="ft0")
            ft1 = cpool.tile([128, BS], _F16, tag="ft1")
            fts = (ft0, ft1)

            def feat_chunk(lo, hi):
                mov = (wxa[:, HID + lo:HID + hi] if hi <= 256
                       else wxb[:, lo - 256:hi - 256])
                for m in range(2):
                    pf = psf.tile([128, hi - lo], _F32, tag="pf")
                    nc.tensor.matmul(pf[:], wxa[:, m * 128:(m + 1) * 128],
                                     mov, start=True, stop=True)
                    nc.scalar.activation(fts[m][:, lo:hi], pf[:],
                                         mybir.ActivationFunctionType.Tanh)

            sy = wcl[0:8, _SY:_SY + NC_MAIN]
            ly = wcl[0:8, _LY:_LY + BS]
            wc0 = wcl[:, _WC0:_WC0 + NC_MAIN]
            wc1 = wcl[:, _WC1:_WC1 + NC_MAIN]

            def fused_mm(po, b):
                bs = slice(b * 128, (b + 1) * 128)
                nc.tensor.matmul(po, ly[:, bs], sy[:], start=True, stop=False)
                nc.tensor.matmul(po, ft0[:, bs], wc0, start=False, stop=False)
                nc.tensor.matmul(po, ft1[:, bs], wc1, start=False, stop=True)

            yt01 = wpool.tile([128, 2 * NQ], _F16, tag="yt01")
            yt23 = wpool.tile([128, 2 * NQ], _F16, tag="yt23")

            def single_combine(pm, b):
                gy = wpool.tile([128, DOF], _F16, tag="gy")
                nc.vector.tensor_copy(gy[:], pm[:, 0:DOF])
                prod = wpool.tile([128, NQ], _F16, tag="prod")
                nc.vector.tensor_mul(
                    prod[:].rearrange("p (t d) -> p t d", d=DOF),
                    pm[:, DOF:DOF + NQ].rearrange("p (t d) -> p t d", d=DOF),
                    gy[:].unsqueeze(1).broadcast_to([128, NQ // DOF, DOF]))
                if b == 0:
                    # absorb pa's DMA-queue tick so the adds below carry
                    # only their same-engine wait (walrus: one wait/inst)
                    pqabs = wpool.tile([1, 1], _F16, tag="pqabs")
                    nc.vector.tensor_copy(pqabs[:], pa_sb[0:1, 0:1])
                nc.vector.tensor_add(yt01[:, b * NQ:(b + 1) * NQ], prod[:],
                                     pa_sb[:, b * NQ:(b + 1) * NQ])

            # emission order == per-engine schedule order: keep tile b0's
            # chain (feat c0/c1 -> pm0 -> combine0) ahead of the later
            # chunks so the DVE starts as early as possible.
            feat_chunk(0, 128)
            feat_chunk(128, 256)
            pm0 = psm.tile([128, NC_MAIN], _F32, tag="pm0")
            fused_mm(pm0[:], 0)
            single_combine(pm0[:], 0)

            # absorb wxb's DMA-queue tick on PE before the c2 matmuls (they
            # already carry a pf-slot-release wait)
            nc.tensor.transpose(pm23[0:1, 1008:1024][:, 1:2],
                                wxb[0:1, 0:2].bitcast(_F32), one_sb[:])
            feat_chunk(256, 512)

            pm1 = psm.tile([128, NC_MAIN], _F32, tag="pm1")
            fused_mm(pm1[:], 1)
            single_combine(pm1[:], 1)
            nc.sync.dma_start(
                y_d[0:256, :].rearrange("(c p) q -> p c q", c=2),
                yt01[:].rearrange("p (c q) -> p c q", q=NQ))

            # tiles 2 and 3: shared 2-bank PSUM tile, batched pair combine;
            # interleave the two tiles' matmuls so both finish right after
            # the last tanh chunk lands.
            po2 = pm23[:, 0:NC_MAIN]
            po3 = pm23[:, 512:512 + NC_MAIN]
            for j in range(2):
                po = (po2, po3)[j]
                bs = slice((2 + j) * 128, (3 + j) * 128)
                nc.tensor.matmul(po, ly[:, bs], sy[:], start=True, stop=False)
            for j in range(2):
                po = (po2, po3)[j]
                bs = slice((2 + j) * 128, (3 + j) * 128)
                nc.tensor.matmul(po, ft0[:, bs], wc0, start=False, stop=False)
            for j in range(2):
                po = (po2, po3)[j]
                bs = slice((2 + j) * 128, (3 + j) * 128)
                nc.tensor.matmul(po, ft1[:, bs], wc1, start=False, stop=True)

            pm3 = pm23[:].rearrange("p (b q) -> p b q", b=2)
            gyp = wpool.tile([128, 2 * DOF], _F16, tag="gyp")
            nc.vector.tensor_copy(
                gyp[:].rearrange("p (b g) -> p b g", b=2), pm3[:, :, 0:DOF])
            prodp = wpool.tile([128, 2 * NQ], _F16, tag="prodp")
            nc.vector.tensor_mul(
                prodp[:].rearrange("p (b t d) -> p b t d", b=2, d=DOF),
                pm3[:, :, DOF:DOF + NQ].rearrange("p b (t d) -> p b t d",
                                                  d=DOF),
                gyp[:].rearrange("p (b g) -> p b g", b=2)
                .unsqueeze(2).broadcast_to([128, 2, NQ // DOF, DOF]))
            nc.vector.tensor_add(yt23[:], prodp[:], pa_sb[:, 2 * NQ:])
            nc.sync.dma_start(
                y_d[256:512, :].rearrange("(c p) q -> p c q", c=2),
                yt23[:].rearrange("p (c q) -> p c q", q=NQ))
    return nc


_NC_CACHE = None

# Optional knobs for local profiling harnesses (defaults are grading-safe).
TRACE = False
LAST_RESULT = None


def _get_nc():
    global _NC_CACHE
    if _NC_CACHE is None:
        _NC_CACHE = _build_nc()
    return _NC_CACHE


def _host_tensors(W_pt, b_pt, W_last, b_last, c, h):
    """Fold scan coefficients into the weight tensors (float64 -> fp16)."""
    G, coef_goal, A, Cst = _precompute_coeffs(c, h)
    W_last = np.asarray(W_last, np.float64)
    b_last = np.asarray(b_last, np.float64)

    # WG[f, q=(t-1)*7+d] = sum_n W_last[f, 7+30d+n] * G[t, n],  t=1..50
    Wr = W_last[:, DOF:].reshape(HID, DOF, N)
    WG = np.einsum("fdn,tn->ftd", Wr, G[1:]).reshape(HID, NQ)
    wc = np.zeros((HID, NC_MAIN))
    wc[:, 0:DOF] = W_last[:, :DOF] * SCALE
    wc[:, DOF:DOF + NQ] = WG * SCALE

    br = b_last[DOF:].reshape(DOF, N)
    bGq = np.einsum("dn,tn->td", br, G[1:]).reshape(NQ) * SCALE

    sy = np.zeros((8, NC_MAIN))
    sy[:DOF, :DOF] = -np.eye(DOF)                  # gy0 = goal - y0
    sy[7, :DOF] = b_last[:DOF] * SCALE
    sy[7, DOF:DOF + NQ] = bGq + np.repeat(coef_goal[1:], DOF)

    # wcl pack [128, 1586]: [wc0 | wc1 | sy | (lyT per core)]
    wcl = np.zeros((128, _WCL_W))
    wcl[:, _WC0:_WC0 + NC_MAIN] = wc[0:128]
    wcl[:, _WC1:_WC1 + NC_MAIN] = wc[128:256]
    wcl[0:8, _SY:_SY + NC_MAIN] = sy

    # wpt_aug [66, 256]: rows 0:64 W_pt, row 64 b_pt, row 65 zero
    wpt_aug = np.zeros((KA, HID))
    wpt_aug[0:D_IN] = np.asarray(W_pt, np.float64)
    wpt_aug[D_IN] = b_pt
    return wcl, wpt_aug, A, Cst


def _build_in_maps(x, state, W_pt, b_pt, W_last, b_last, c, h):
    x = np.asarray(x, np.float64)
    state = np.asarray(state, np.float64)
    wcl_shared, wpt_aug, A, Cst = _host_tensors(W_pt, b_pt, W_last, b_last,
                                                c, h)

    # pa[i, (t-1)*7+d] = A[t]*y0[i,d] + Cst[t],  t=1..50
    pa_full = (np.repeat(A[1:], DOF)[None, :]
               * np.tile(state, (1, NT - 1))
               + np.repeat(Cst[1:], DOF)[None, :])        # [B, 350]

    xT_aug = np.zeros((KA, B))
    xT_aug[0:D_IN] = x.T
    xT_aug[D_IN] = 1.0
    in_maps = []
    for i in range(NCORES):
        sl = slice(i * BS, (i + 1) * BS)
        xs = xT_aug[:, sl]
        wxa = np.concatenate([wpt_aug, xs[:, 0:256]], axis=1)   # [66, 512]
        wxb = xs[:, 256:BS]                                     # [66, 256]

        wcl = wcl_shared.copy()
        wcl[0:DOF, _LY:_LY + BS] = state[sl].T
        wcl[DOF, _LY:_LY + BS] = 1.0

        pa_c = pa_full[sl]                                # [512, 350]
        pa_s = np.concatenate([pa_c[j * 128:(j + 1) * 128] for j in range(4)],
                              axis=1)                     # [128, 1400]

        in_maps.append({
            "wxa_s": np.ascontiguousarray(wxa, np.float16),
            "wxb_s": np.ascontiguousarray(wxb, np.float16),
            "wcl_s": np.ascontiguousarray(wcl, np.float16),
            "pa_s": np.ascontiguousarray(pa_s, np.float16),
        })
    return in_maps


def kernel(x, state, W_pt, b_pt, W_last, b_last, c, h):
    in_maps = _build_in_maps(x, state, W_pt, b_pt, W_last, b_last, c, h)
    nc = _get_nc()
    global LAST_RESULT
    LAST_RESULT = run_bass_kernel_spmd(nc, in_maps, list(range(NCORES)),
                                       trace=TRACE)
    res = LAST_RESULT.results
    yq = np.concatenate([r["y"] for r in res], axis=0)    # [B, 350] fp16
    out = np.empty((B, NT, DOF), np.float32)
    out[:, 0, :] = np.asarray(state, np.float32)
    out[:, 1:, :] = yq.astype(np.float32).reshape(B, NT - 1, DOF)
    return out


def kernel_sim(x, state, W_pt, b_pt, W_last, b_last, c, h, core=0):
    """CoreSim a single core's shard; returns (y_shard [BS,NT,DOF], sim)."""
    from concourse.bass_interp import CoreSim
    in_maps = _build_in_maps(x, state, W_pt, b_pt, W_last, b_last, c, h)
    sim = CoreSim(_build_nc(), publish_trace=False)
    for k, v in in_maps[core].items():
        sim.tensor(k)[:] = v
    sim.simulate()
    yq = np.array(sim.tensor("y"))
    out = np.empty((BS, NT, DOF), np.float32)
    out[:, 0, :] = np.asarray(state, np.float32)[core * BS:(core + 1) * BS]
    out[:, 1:, :] = yq.astype(np.float32).reshape(BS, NT - 1, DOF)
    return out, sim

